# revision 18
# baseline (speedup 1.0000x reference)
"""GRU kernel for Trainium2 (Bass/Tile), 8-core batch-sharded SPMD.

Problem: B=64, T=512, I=128, R=512, O=64 GRU with per-step noise.
  r = sigmoid(x_t @ Wr_x.T + h @ Wr_h.T + br)
  z = sigmoid(x_t @ Wz_x.T + h @ Wz_h.T + bz)
  n = tanh(x_t @ Wn_x.T + bn_x + r * (h @ Wn_h.T + bn_h))
  h' = (1-z)*n + z*h + noise_t
Outputs: y = hstore @ Wy.T + by  [B,T,O],  hstore [B,T,R]

Design notes:
 - Data-parallel over batch: 8 cores x 8 sequences.
 - Recurrent matmuls in "form B": stationary lhsT = h.T tiles [128,8],
   moving rhs = W.T chunks [128,512] as float32r (1 cycle/row at N>=512).
   Gate psums land [8,512]; x-side projections are folded in as one extra
   matmul per gate with x_t.T stationary (no precompute pass).
 - h' is re-transposed each step via 4 PE transpose ops into a [128,32]
   tile that is both the next step's lhsT and the y-GEMM moving operand.
 - y computed in-loop every YB steps from the hT ring; outputs streamed.
"""

import numpy as np

import concourse.bass as bass
import concourse.bacc as bacc
import concourse.mybir as mybir
from concourse import tile
from concourse.bass_utils import run_bass_kernel_spmd

B, T, I, R, O = 64, 512, 128, 512, 64
NCORES = 8
BL = B // NCORES          # 8 sequences per core
K = R // 128              # 4 k-tiles of the recurrent contraction
NB = 8                    # steps per hstore/noise DMA block
YB = 32                   # steps per y-GEMM block
NYB = T // YB

f32 = mybir.dt.float32
f32r = mybir.dt.float32r
AF = mybir.ActivationFunctionType
ALU = mybir.AluOpType


def _build_program(has_b, T_steps=T):
    """Build the single-core SPMD program. has_b: dict of which bias
    vectors are nonzero (emits exact K=1 bias matmuls only when needed)."""
    nc = bacc.Bacc("TRN2", target_bir_lowering=False, debug=False,
                   num_devices=NCORES)

    # ---- DRAM I/O (per core slices, host-prepped layouts) ----
    xT_d = nc.dram_tensor("xT", [I, T_steps, BL], f32r, kind="ExternalInput")
    noise_d = nc.dram_tensor("noise", [BL, T_steps, R], f32, kind="ExternalInput")
    WhT_d = nc.dram_tensor("WhT", [128, 3 * K * R], f32r, kind="ExternalInput")
    WxT_d = nc.dram_tensor("WxT", [128, 3 * R], f32r, kind="ExternalInput")
    WyT_d = nc.dram_tensor("WyT", [128, K * O], f32r, kind="ExternalInput")
    ident_d = nc.dram_tensor("ident", [BL, BL], f32, kind="ExternalInput")
    bias_d = nc.dram_tensor("biasrows", [1, 4 * R], f32r, kind="ExternalInput")
    ones_d = nc.dram_tensor("ones", [1, BL], f32r, kind="ExternalInput")
    by_d = nc.dram_tensor("by", [O, 1], f32, kind="ExternalInput")

    hstore_d = nc.dram_tensor("hstore", [BL, T_steps, R], f32, kind="ExternalOutput")
    nyb = T_steps // YB
    yT_d = nc.dram_tensor("yT", [nyb, O, YB * BL], f32, kind="ExternalOutput")

    # ---- persistent SBUF ----
    WhT_sb = nc.alloc_sbuf_tensor("WhT_sb", [128, 3 * K * R], f32r).ap()
    WxT_sb = nc.alloc_sbuf_tensor("WxT_sb", [128, 3 * R], f32r).ap()
    WyT_sb = nc.alloc_sbuf_tensor("WyT_sb", [128, K * O], f32r).ap()
    xT_sb = nc.alloc_sbuf_tensor("xT_sb", [128, T_steps * BL], f32r).ap()
    ident_sb = nc.alloc_sbuf_tensor("ident_sb", [BL, BL], f32).ap()
    bias_sb = nc.alloc_sbuf_tensor("bias_sb", [1, 4 * R], f32r).ap()
    ones_sb = nc.alloc_sbuf_tensor("ones_sb", [1, BL], f32r).ap()
    by_sb = nc.alloc_sbuf_tensor("by_sb", [O, 1], f32).ap()
    # noise + h staging rings: [BL, parity, NB, R]
    noise_rg = nc.alloc_sbuf_tensor("noise_rg", [BL, 2 * NB * R], f32).ap()
    h_rg = nc.alloc_sbuf_tensor("h_rg", [BL, 2 * NB * R], f32).ap()
    # transposed-h ring, depth YB steps: free = (t % YB, j, b)
    hT_rg = nc.alloc_sbuf_tensor("hT_rg", [128, YB * K * BL], f32r).ap()
    ystage = nc.alloc_sbuf_tensor("ystage", [O, 2 * YB * BL], f32).ap()

    def wh(g, k):       # W{g}_h.T chunk [128, R] for k-tile k
        return WhT_sb[:, (g * K + k) * R:(g * K + k + 1) * R]

    def wx(g):          # W{g}_x.T [128, R]
        return WxT_sb[:, g * R:(g + 1) * R]

    def xt(t):          # x_t.T stationary [128, BL]
        return xT_sb[:, t * BL:(t + 1) * BL]

    def hT_at(t):       # lhsT tile source for step t reads h' of step t-1
        s = t % YB
        return hT_rg[:, s * K * BL:(s + 1) * K * BL]

    def ring(buf, t):   # [BL, R] slot in a [BL, 2*NB*R] ring
        s = t % (2 * NB)
        return buf[:, s * R:(s + 1) * R]

    with tile.TileContext(nc) as tc:
        with (
            tc.tile_pool(name="gates", bufs=1, space="PSUM") as gpool,
            tc.tile_pool(name="hTp", bufs=2, space="PSUM") as hTpool,
            tc.tile_pool(name="yp", bufs=1, space="PSUM") as ypool,
            tc.tile_pool(name="temps", bufs=3) as temps,
        ):
            # ---- preload ----
            nc.sync.dma_start(WhT_sb, WhT_d.ap())
            nc.sync.dma_start(WxT_sb, WxT_d.ap())
            nc.sync.dma_start(WyT_sb, WyT_d.ap())
            nc.sync.dma_start(xT_sb, xT_d.ap().rearrange("i t b -> i (t b)"))
            nc.sync.dma_start(ident_sb, ident_d.ap())
            nc.sync.dma_start(bias_sb, bias_d.ap())
            nc.sync.dma_start(ones_sb, ones_d.ap())
            nc.sync.dma_start(by_sb, by_d.ap())
            # initial state: h(-1) = 0 (hT for t=0 is skipped entirely;
            # memset on a float32r region fails the ISA check)
            nc.vector.memset(ring(h_rg, -1), 0.0)
            # first noise block
            nc.sync.dma_start(
                noise_rg[:, 0:NB * R],
                noise_d.ap()[:, 0:NB, :].rearrange("b t r -> b (t r)"))

            for t in range(T_steps):
                hT_prev = hT_at(t)
                # ---------- gate matmuls ----------
                # order: z, r, n, xn  (z needed early for p/p2; r right
                # before the n-gate tail)
                psums = {}
                hmm = t > 0   # step 0 has h(-1)=0: skip the h-side matmuls
                for g, gname in ((1, "z"), (0, "r")):
                    ps = gpool.tile([BL, R], f32, tag=gname)
                    psums[gname] = ps
                    if hmm:
                        for k in range(K):
                            nc.tensor.matmul(
                                ps[:], hT_prev[:, k * BL:(k + 1) * BL],
                                wh(g, k), start=(k == 0), stop=False)
                    last = not has_b[gname]
                    nc.tensor.matmul(ps[:], xt(t), wx(g),
                                     start=not hmm, stop=last)
                    if has_b[gname]:
                        nc.tensor.matmul(ps[:], ones_sb,
                                         bias_sb[:, g * R:(g + 1) * R],
                                         start=False, stop=True)
                ps_n = gpool.tile([BL, R], f32, tag="n")
                if hmm:
                    for k in range(K):
                        nc.tensor.matmul(
                            ps_n[:], hT_prev[:, k * BL:(k + 1) * BL],
                            wh(2, k), start=(k == 0),
                            stop=(k == K - 1 and not has_b["bnh"]))
                    if has_b["bnh"]:
                        nc.tensor.matmul(ps_n[:], ones_sb,
                                         bias_sb[:, 3 * R:4 * R],
                                         start=False, stop=True)
                elif has_b["bnh"]:
                    nc.tensor.matmul(ps_n[:], ones_sb, bias_sb[:, 3 * R:4 * R],
                                     start=True, stop=True)
                else:
                    nc.vector.memset(ps_n[:], 0.0)
                ps_xn = gpool.tile([BL, R], f32, tag="xn")
                nc.tensor.matmul(ps_xn[:], xt(t), wx(2),
                                 start=True, stop=not has_b["bnx"])
                if has_b["bnx"]:
                    nc.tensor.matmul(ps_xn[:], ones_sb, bias_sb[:, 2 * R:3 * R],
                                     start=False, stop=True)

                # ---------- gate nonlinearities / combine ----------
                z_sb = temps.tile([BL, R], f32, tag="z_sb")
                r_sb = temps.tile([BL, R], f32, tag="r_sb")
                nc.scalar.activation(z_sb[:], psums["z"][:], AF.Sigmoid)
                nc.scalar.activation(r_sb[:], psums["r"][:], AF.Sigmoid)

                h_prev = ring(h_rg, t - 1)
                nz_t = ring(noise_rg, t)
                p = temps.tile([BL, R], f32, tag="p")
                p2 = temps.tile([BL, R], f32, tag="p2")
                nc.gpsimd.tensor_tensor(p[:], z_sb[:], h_prev, op=ALU.mult)
                nc.gpsimd.tensor_tensor(p2[:], p[:], nz_t, op=ALU.add)

                v = temps.tile([BL, R], f32, tag="v")
                w = temps.tile([BL, R], f32, tag="w")
                nc.vector.tensor_tensor(v[:], r_sb[:], ps_n[:], op=ALU.mult)
                nc.vector.tensor_tensor(w[:], v[:], ps_xn[:], op=ALU.add)
                n_sb = temps.tile([BL, R], f32, tag="n_sb")
                nc.scalar.activation(n_sb[:], w[:], AF.Tanh)

                q = temps.tile([BL, R], f32, tag="q")
                nc.vector.scalar_tensor_tensor(
                    q[:], z_sb[:], 1.0, n_sb[:], op0=ALU.subtract, op1=ALU.mult)
                h_new = ring(h_rg, t)
                nc.vector.tensor_tensor(h_new, p2[:], q[:], op=ALU.subtract)

                # ---------- transpose h' for next step ----------
                psT = hTpool.tile([128, K * BL], f32, tag="hT")
                for j in range(K):
                    nc.tensor.transpose(
                        psT[:, j * BL:(j + 1) * BL],
                        h_new[:, j * 128:(j + 1) * 128], ident_sb)
                hT_next = hT_at(t + 1)
                nc.scalar.activation(hT_next, psT[:], AF.Copy, bias=0.0)

                # ---------- streaming I/O ----------
                if t % NB == NB - 1:
                    blk = t // NB
                    lo = (blk % 2) * NB * R
                    nc.sync.dma_start(
                        hstore_d.ap()[:, blk * NB:(blk + 1) * NB, :]
                        .rearrange("b t r -> b (t r)"),
                        h_rg[:, lo:lo + NB * R])
                    if (blk + 1) * NB < T_steps:
                        lo2 = ((blk + 1) % 2) * NB * R
                        nc.sync.dma_start(
                            noise_rg[:, lo2:lo2 + NB * R],
                            noise_d.ap()[:, (blk + 1) * NB:(blk + 2) * NB, :]
                            .rearrange("b t r -> b (t r)"))

                # ---------- y drain every YB steps ----------
                if t % YB == YB - 1:
                    yblk = t // YB
                    ps_y = ypool.tile([O, YB * BL], f32, tag="y")
                    for j in range(K):
                        # moving rhs: hT ring, free dims (t:YB, b:BL) at
                        # k-tile j; steps (t+1)%YB order == write order s
                        rhs = hT_rg.rearrange(
                            "p (s k b) -> p s k b", k=K, b=BL)[:, :, j, :]

                        nc.tensor.matmul(
                            ps_y[:], WyT_sb[:, j * O:(j + 1) * O], rhs,
                            start=(j == 0), stop=(j == K - 1))
                    ylo = (yblk % 2) * YB * BL
                    yst = ystage[:, ylo:ylo + YB * BL]
                    nc.scalar.activation(yst, ps_y[:], AF.Identity, bias=by_sb)
                    nc.sync.dma_start(yT_d.ap()[yblk], yst)

    nc.compile()
    return nc


def _prep_inputs(x, noise, Wr_x, Wr_h, br, Wz_x, Wz_h, bz,
                 Wn_x, bn_x, Wn_h, bn_h, Wy, by):
    """Host-side layout prep (pure layout transforms, no math)."""
    # WhT[p, (g,k,r)] = Wg_h[r, 128k+p]
    WhT = np.ascontiguousarray(
        np.stack([Wr_h.T.reshape(K, 128, R), Wz_h.T.reshape(K, 128, R),
                  Wn_h.T.reshape(K, 128, R)])          # [3, K, 128, R]
        .transpose(2, 0, 1, 3).reshape(128, 3 * K * R))
    WxT = np.ascontiguousarray(
        np.stack([Wr_x.T, Wz_x.T, Wn_x.T])             # [3, 128, R]
        .transpose(1, 0, 2).reshape(128, 3 * R))
    WyT = np.ascontiguousarray(
        Wy.T.reshape(K, 128, O).transpose(1, 0, 2).reshape(128, K * O))
    ident = np.eye(BL, dtype=np.float32)
    biasrows = np.ascontiguousarray(
        np.stack([br, bz, bn_x, bn_h]).reshape(1, 4 * R))
    ones = np.ones((1, BL), np.float32)
    byc = np.ascontiguousarray(by.reshape(O, 1))

    in_maps = []
    for c in range(NCORES):
        sl = slice(c * BL, (c + 1) * BL)
        xc = x[sl]                                   # [BL, T, I]
        xT = np.ascontiguousarray(xc.transpose(2, 1, 0))  # [I, T, BL]
        in_maps.append({
            "xT": xT,
            "noise": np.ascontiguousarray(noise[sl]),
            "WhT": WhT, "WxT": WxT, "WyT": WyT,
            "ident": ident, "biasrows": biasrows, "ones": ones, "by": byc,
        })
    return in_maps


def kernel(x, noise, Wr_x, Wr_h, br, Wz_x, Wz_h, bz,
           Wn_x, bn_x, Wn_h, bn_h, Wy, by, _trace=False):
    x = np.asarray(x); noise = np.asarray(noise)
    args = dict(x=x, noise=noise, Wr_x=np.asarray(Wr_x), Wr_h=np.asarray(Wr_h),
                br=np.asarray(br), Wz_x=np.asarray(Wz_x), Wz_h=np.asarray(Wz_h),
                bz=np.asarray(bz), Wn_x=np.asarray(Wn_x), bn_x=np.asarray(bn_x),
                Wn_h=np.asarray(Wn_h), bn_h=np.asarray(bn_h),
                Wy=np.asarray(Wy), by=np.asarray(by))
    has_b = {
        "r": bool(np.any(args["br"])), "z": bool(np.any(args["bz"])),
        "bnx": bool(np.any(args["bn_x"])), "bnh": bool(np.any(args["bn_h"])),
    }
    nc = _build_program(has_b)
    in_maps = _prep_inputs(**args)
    kw = {}
    if _trace:
        import os
        kw = dict(trace=True, tmpdir=os.path.abspath("trace_out"))
        os.makedirs(kw["tmpdir"], exist_ok=True)
    res = run_bass_kernel_spmd(nc, in_maps, list(range(NCORES)), **kw)

    hstore = np.concatenate([res.results[c]["hstore"] for c in range(NCORES)])
    y = np.empty((B, T, O), np.float32)
    for c in range(NCORES):
        yT = res.results[c]["yT"]                    # [NYB, O, YB*BL]
        # free index f = s*BL + b where s = (t+1) % YB within block
        yc = yT.reshape(NYB, O, YB, BL)              # [blk, o, s, b]
        # step t in block: s = (t % YB + 1) % YB -> t = (s - 1) % YB
        yc = np.roll(yc, -1, axis=2)                 # now s aligned to t
        y[c * BL:(c + 1) * BL] = yc.transpose(3, 0, 2, 1).reshape(BL, T, O)
    if _trace:
        kernel._last_results = res
    return y, hstore
